# revision 7
# baseline (speedup 1.0000x reference)
"""Trainium2 Bass kernel for BasicTransformerBlock_Epipolar (relative-position attention).

Math (per batch b):
  q = x@Wq, k = x@Wk, v = x@Wv            (heads H=16, dh=64)
  sim[t,s]  = (q_h[t]·k_h[s] + q_h[t]·Tk[s-t+1024]) * dh^-0.5
  attn      = softmax_s(sim)
  out[t]    = sum_s attn[t,s]*(v_h[s]) + sum_s attn[t,s]*Tv[s-t+1024]
  y = out@Wo + bo

Sharding: data-parallel over batch, 2 batches per core, 8 cores, no collectives.

The relative-position terms need a "skew" (diagonal remap) which SBUF access
patterns cannot express; both are routed through DRAM with strided access
patterns:
  - R[t,r] = q[t]·Tk[r] is computed blockwise as a plain matmul, written to a
    row-stride-2049 buffer, and read back with row stride 2048, which yields
    exactly M[t,s] = R[t, s-t+1024].
  - attn is written contiguously (row stride 1024) and read back with row
    stride 1025 (+ xbar transpose), which yields A_skew^T[j,i] =
    attn[i, i+j-127]; out_rel^T = sum_j Tv_win[j]·A_skew^T[j] accumulates into
    the same PSUM as attn@v.
"""

import sys

sys.path.insert(0, "/opt/trn_rl_repo")

import numpy as np

import concourse.bass as bass
import concourse.tile as tile
from concourse import bacc, mybir
from concourse.bass_utils import run_bass_kernel_spmd
from concourse.masks import make_identity

FP = mybir.dt.float32
BF = mybir.dt.bfloat16

B, T, D = 16, 1024, 1024
H, DH = 16, 64
NCORE = 8
BL = B // NCORE          # batches per core
TL = BL * T              # local token rows
SCALE = DH ** -0.5
NHP = H // 2             # head pairs
NBLK = T // 128          # 128-row blocks per batch
WREL = 1151              # rel window width per 128 t-block
RSTRIDE = 2049
RSEG = T * RSTRIDE       # rbuf elements per (b, h)
AGUARD = 128
ASEG = AGUARD + 128 * 1024 + AGUARD  # abuf elements per (b, h, blk)


def _ap(t_ap, offset, pattern):
    return bass.AP(tensor=t_ap.tensor, offset=offset, ap=pattern)


def build(num_b=BL, num_hp=NHP, num_blk=NBLK):
    nc = bacc.Bacc("TRN2", target_bir_lowering=False, debug=False, num_devices=NCORE)

    x = nc.dram_tensor("x", [BL, T, D], FP, kind="ExternalInput").ap()
    wq = nc.dram_tensor("Wq", [D, D], FP, kind="ExternalInput").ap()
    wk = nc.dram_tensor("Wk", [D, D], FP, kind="ExternalInput").ap()
    wv = nc.dram_tensor("Wv", [D, D], FP, kind="ExternalInput").ap()
    wo = nc.dram_tensor("Wo", [D, D], FP, kind="ExternalInput").ap()
    bo = nc.dram_tensor("bo", [D], FP, kind="ExternalInput").ap()
    tk = nc.dram_tensor("rel_k_table", [2 * T + 1, DH], FP, kind="ExternalInput").ap()
    tv = nc.dram_tensor("rel_v_table", [2 * T + 1, DH], FP, kind="ExternalInput").ap()
    y = nc.dram_tensor("y", [BL, T, D], FP, kind="ExternalOutput").ap()

    qT = nc.dram_tensor("qT", [D, TL], FP).ap()
    kT = nc.dram_tensor("kT", [D, TL], FP).ap()
    vB = nc.dram_tensor("vB", [TL, D], FP).ap()
    aot = nc.dram_tensor("aot", [D, TL], FP).ap()
    rbufs = [
        nc.dram_tensor(f"rbuf{h}", [BL * NHP * RSEG], FP).ap() for h in (0, 1)
    ]
    abuf = nc.dram_tensor("abuf", [BL * H * NBLK * ASEG], BF).ap()

    with tile.TileContext(nc) as tc:
        const = tc.alloc_tile_pool(name="const", bufs=1)
        ps512 = tc.alloc_tile_pool(name="ps512", bufs=4, space="PSUM")
        pst = tc.alloc_tile_pool(name="pst", bufs=2, space="PSUM")
        pso = tc.alloc_tile_pool(name="pso", bufs=2, space="PSUM")

        # ---- constants ----
        ident = const.tile([128, 128], FP, tag="ident")
        make_identity(nc, ident)

        ones_u8 = const.tile([128, 128], mybir.dt.int8, tag="ones_u8")
        nc.vector.memset(ones_u8, 1)
        zeros_bf = const.tile([128, 128], BF, tag="zeros_bf")
        nc.vector.memset(zeros_bf, 0.0)
        # mask0[p,f] = 1 if p+f >= 127 ; mask8[p,f] = 1 if p+f <= 126
        mask0 = const.tile([128, 128], mybir.dt.int8, tag="mask0")
        nc.gpsimd.affine_select(
            out=mask0, in_=ones_u8, pattern=[[1, 128]],
            compare_op=mybir.AluOpType.is_ge, fill=0, base=-127,
            channel_multiplier=1,
        )
        mask8 = const.tile([128, 128], mybir.dt.int8, tag="mask8")
        nc.gpsimd.affine_select(
            out=mask8, in_=ones_u8, pattern=[[-1, 128]],
            compare_op=mybir.AluOpType.is_ge, fill=0, base=126,
            channel_multiplier=-1,
        )

        # bo broadcast to all partitions
        bo128 = const.tile([128, D], FP, tag="bo128")
        nc.sync.dma_start(out=bo128, in_=_ap(bo, 0, [[0, 128], [1, D]]))

        # Tk^T resident in SBUF, duplicated across both partition halves so it
        # can feed row-packed (tile_position) matmuls for either head.
        # tkT[p, r] = tk[r, p%64]  for r in [0, 2048)
        tk_tmp = const.tile([128, 16, DH], FP, tag="tk_tmp")
        nc.sync.dma_start(
            out=tk_tmp, in_=tk[0 : 16 * 128, :].rearrange("(c p) d -> p c d", p=128)
        )
        tkT = const.tile([128, 16 * 128], FP, tag="tkT")
        for c in range(16):
            ptile = pst.tile([128, 128], FP, tag="pst")
            nc.tensor.transpose(ptile[0:DH, :], tk_tmp[:, c, :], ident)
            nc.scalar.copy(out=tkT[0:DH, c * 128 : (c + 1) * 128], in_=ptile[0:DH, :])
            nc.scalar.copy(out=tkT[DH:128, c * 128 : (c + 1) * 128], in_=ptile[0:DH, :])

        # Tv rows 1..2048 as bf16 chunks: tv_bf[p, m, d] = tv[1 + 128m + p, d]
        tv_tmp = const.tile([128, 16, DH], FP, tag="tv_tmp")
        nc.sync.dma_start(
            out=tv_tmp, in_=tv[1 : 1 + 16 * 128, :].rearrange("(m p) d -> p m d", p=128)
        )
        tv_bf = const.tile([128, 16, DH], BF, tag="tv_bf")
        nc.scalar.copy(out=tv_bf, in_=tv_tmp)

        # ================= Phase A: projections =================
        pa = tc.alloc_tile_pool(name="pa", bufs=2)
        pev = tc.alloc_tile_pool(name="pev", bufs=4)

        # x^T resident: xT[p, ic, tg] = x[tg//T, tg%T, 128*ic + p]
        xT = pa.tile([128, 8, TL], FP, tag="xT", bufs=1)
        for b in range(num_b):
            for tb in range(8):
                xt = pa.tile([128, D], FP, tag="xt")
                nc.sync.dma_start(out=xt, in_=x[b, tb * 128 : (tb + 1) * 128, :])
                for ic in range(8):
                    ptile = pst.tile([128, 128], FP, tag="pst")
                    nc.tensor.transpose(ptile, xt[:, ic * 128 : (ic + 1) * 128], ident)
                    nc.scalar.copy(
                        out=xT[:, ic, (b * 8 + tb) * 128 : (b * 8 + tb + 1) * 128],
                        in_=ptile,
                    )

        # q^T, k^T  (dst[j, tg] = sum_i W[i, j] * x[tg, i])
        for w_ap, dst in ((wq, qT), (wk, kT)):
            wsb = pa.tile([128, 8, D], FP, tag="wsb")
            nc.sync.dma_start(out=wsb, in_=w_ap.rearrange("(c p) j -> p c j", p=128))
            for jt in range(8):
                for tt in range(num_b * 2):
                    ps = ps512.tile([128, 512], FP, tag="ps512")
                    for ic in range(8):
                        nc.tensor.matmul(
                            ps,
                            lhsT=wsb[:, ic, jt * 128 : (jt + 1) * 128],
                            rhs=xT[:, ic, tt * 512 : (tt + 1) * 512],
                            start=(ic == 0),
                            stop=(ic == 7),
                        )
                    ev = pev.tile([128, 512], FP, tag="ev")
                    nc.scalar.copy(out=ev, in_=ps)
                    nc.sync.dma_start(
                        out=dst[jt * 128 : (jt + 1) * 128, tt * 512 : (tt + 1) * 512],
                        in_=ev,
                    )

        # v (natural layout): vB[tg, j] = sum_i x[tg, i] * Wv[i, j]
        wsb = pa.tile([128, 8, D], FP, tag="wsb")
        nc.sync.dma_start(out=wsb, in_=wv.rearrange("(c p) j -> p c j", p=128))
        for tt in range(num_b * 8):
            for jh in range(2):
                ps = ps512.tile([128, 512], FP, tag="ps512")
                for ic in range(8):
                    nc.tensor.matmul(
                        ps,
                        lhsT=xT[:, ic, tt * 128 : (tt + 1) * 128],
                        rhs=wsb[:, ic, jh * 512 : (jh + 1) * 512],
                        start=(ic == 0),
                        stop=(ic == 7),
                    )
                ev = pev.tile([128, 512], FP, tag="ev")
                nc.scalar.copy(out=ev, in_=ps)
                nc.sync.dma_start(
                    out=vB[tt * 128 : (tt + 1) * 128, jh * 512 : (jh + 1) * 512],
                    in_=ev,
                )

        pev.release()
        pa.release()

        # ================= Phase B: attention =================
        pb = tc.alloc_tile_pool(name="pb", bufs=2)

        def stage1(b, hp, k, qk, kt, vv):
            """R matmuls + skew bounce + sim + softmax + abuf write.

            Returns context for stage2."""
            t0 = 128 * k
            r0 = 897 - t0
            attns = []
            segs = []
            for h in (0, 1):
                hg = 2 * hp + h
                rbuf = rbufs[h]
                bh_base = (b * NHP + hp) * RSEG
                seg = ((b * H + hg) * NBLK + k) * ASEG
                segs.append(seg)
                tp = (64 * h, 0)
                lhs_q = qk[64 * h : 64 * h + 64, t0 : t0 + 128]

                # R[t, r] = q[t]·Tk[r] over the block window, bounced via DRAM
                rsb = pb.tile([128, WREL], FP, tag=f"rsb{h}")
                for c0, cw in ((0, 512), (512, 512), (1024, 127)):
                    ps = ps512.tile([128, 512], FP, tag="ps512")
                    nc.tensor.matmul(
                        ps[:, 0:cw],
                        lhsT=lhs_q,
                        rhs=tkT[64 * h : 64 * h + 64, r0 + c0 : r0 + c0 + cw],
                        start=True,
                        stop=True,
                        tile_position=tp,
                    )
                    nc.scalar.copy(out=rsb[:, c0 : c0 + cw], in_=ps[:, 0:cw])
                nc.sync.dma_start(
                    out=_ap(rbuf, bh_base + t0 * 2048 + 897, [[2049, 128], [1, WREL]]),
                    in_=rsb,
                )

                # skewed read-back: msk[i, s] = R[t0+i, s - (t0+i) + 1024]
                msk = pb.tile([128, T], FP, tag=f"msk{h}")
                nc.sync.dma_start(
                    out=msk,
                    in_=_ap(rbuf, bh_base + t0 * 2048 + 1024, [[2048, 128], [1, T]]),
                )

                # sim = q@k^T, then attn = exp(SCALE*(sim+msk)) / rowsum
                attn = pb.tile([128, T], FP, tag=f"attn{h}")
                for n in range(2):
                    ps = ps512.tile([128, 512], FP, tag="ps512")
                    nc.tensor.matmul(
                        ps,
                        lhsT=lhs_q,
                        rhs=kt[64 * h : 64 * h + 64, n * 512 : (n + 1) * 512],
                        start=True,
                        stop=True,
                        tile_position=tp,
                    )
                    nc.vector.tensor_add(
                        attn[:, n * 512 : (n + 1) * 512], ps, msk[:, n * 512 : (n + 1) * 512]
                    )
                lsum = pb.tile([128, 1], FP, tag=f"lsum{h}")
                nc.scalar.activation(
                    out=attn, in_=attn, func=mybir.ActivationFunctionType.Exp,
                    scale=float(SCALE), accum_out=lsum,
                )
                recip = pb.tile([128, 1], FP, tag=f"recip{h}")
                nc.vector.reciprocal(recip, lsum)
                nc.vector.tensor_scalar_mul(attn, attn, recip)

                abf = pb.tile([128, T], BF, tag=f"abf{h}")
                nc.scalar.copy(out=abf, in_=attn)
                nc.sync.dma_start(
                    out=_ap(abuf, seg + AGUARD, [[1024, 128], [1, 1024]]), in_=abf
                )
                attns.append(attn)
            return dict(b=b, hp=hp, k=k, vv=vv, attns=attns, segs=segs)

        def stage2(b, hp, k, vv, attns, segs):
            """attn transposes + attn@v + rel_v, accumulate out^T, evict to aot."""
            ats = []
            for h in (0, 1):
                at = pb.tile([128, 8, 128], FP, tag=f"at{h}")
                for c in range(8):
                    ptile = pst.tile([128, 128], FP, tag="pst")
                    nc.tensor.transpose(
                        ptile, attns[h][:, c * 128 : (c + 1) * 128], ident
                    )
                    nc.scalar.copy(out=at[:, c, :], in_=ptile)
                ats.append(at)

            po = pso.tile([128, 128], FP, tag="po")
            for c in range(8):
                for h in (0, 1):
                    nc.tensor.matmul(
                        po[64 * h : 64 * h + 64, :],
                        lhsT=vv[:, c, 64 * h : 64 * h + 64],
                        rhs=ats[h][:, c, :],
                        start=(c == 0),
                        stop=False,
                        tile_position=(0, 64 * h),
                    )
            for c in range(9):
                m = 7 - k + c
                for h in (0, 1):
                    ask = pb.tile([128, 128], BF, tag=f"ask{h}")
                    nc.sync.dma_start_transpose(
                        ask, _ap(abuf, segs[h] + 1 + 128 * c, [[1025, 128], [1, 128]])
                    )
                    if c == 0:
                        askm = pb.tile([128, 128], BF, tag=f"askm{h}")
                        nc.vector.select(askm, mask0, ask, zeros_bf)
                        ask = askm
                    elif c == 8:
                        askm = pb.tile([128, 128], BF, tag=f"askm{h}")
                        nc.vector.select(askm, mask8, ask, zeros_bf)
                        ask = askm
                    nc.tensor.matmul(
                        po[64 * h : 64 * h + 64, :],
                        lhsT=tv_bf[:, m, :],
                        rhs=ask,
                        start=False,
                        stop=(c == 8),
                        tile_position=(0, 64 * h),
                    )
            ot = pb.tile([128, 128], FP, tag="ot")
            nc.scalar.copy(out=ot, in_=po)
            nc.sync.dma_start(
                out=aot[128 * hp : 128 * (hp + 1), b * T + 128 * k : b * T + 128 * (k + 1)],
                in_=ot,
            )

        pending = None
        for b in range(num_b):
            for hp in range(num_hp):
                qk = pb.tile([128, T], FP, tag="qk")
                nc.sync.dma_start(
                    out=qk, in_=qT[128 * hp : 128 * (hp + 1), b * T : (b + 1) * T]
                )
                kt = pb.tile([128, T], FP, tag="kt")
                nc.sync.dma_start(
                    out=kt, in_=kT[128 * hp : 128 * (hp + 1), b * T : (b + 1) * T]
                )
                vv = pb.tile([128, 8, 128], FP, tag="vv")
                nc.sync.dma_start(
                    out=vv,
                    in_=vB[b * T : (b + 1) * T, 128 * hp : 128 * (hp + 1)].rearrange(
                        "(c p) d -> p c d", p=128
                    ),
                )
                for k in range(num_blk):
                    cur = stage1(b, hp, k, qk, kt, vv)
                    if pending is not None:
                        stage2(**pending)
                    pending = cur
        if pending is not None:
            stage2(**pending)
        pb.release()

        # ================= Phase C: output projection =================
        pc = tc.alloc_tile_pool(name="pc", bufs=2)
        wsb_o = pc.tile([128, 8, D], FP, tag="wsb_o", bufs=1)
        nc.sync.dma_start(out=wsb_o, in_=wo.rearrange("(c p) j -> p c j", p=128))
        for tt in range(num_b * 8):
            asb = pc.tile([128, 8, 128], FP, tag="asb")
            nc.sync.dma_start(
                out=asb,
                in_=aot[:, tt * 128 : (tt + 1) * 128].rearrange(
                    "(c p) t -> p c t", p=128
                ),
            )
            for eh in range(2):
                ps = ps512.tile([128, 512], FP, tag="ps512")
                for ic in range(8):
                    nc.tensor.matmul(
                        ps,
                        lhsT=asb[:, ic, :],
                        rhs=wsb_o[:, ic, eh * 512 : (eh + 1) * 512],
                        start=(ic == 0),
                        stop=(ic == 7),
                    )
                ysb = pc.tile([128, 512], FP, tag="ysb")
                nc.vector.tensor_add(ysb, ps, bo128[:, eh * 512 : (eh + 1) * 512])
                nc.sync.dma_start(
                    out=y[tt // 8, (tt % 8) * 128 : (tt % 8 + 1) * 128,
                          eh * 512 : (eh + 1) * 512],
                    in_=ysb,
                )
        pc.release()

        pso.release()
        pst.release()
        ps512.release()
        const.release()

    nc.compile()
    return nc


_NC_CACHE = None


def _make_in_maps(x, Wq, Wk, Wv, Wo, bo, rel_k_table, rel_v_table):
    f32 = lambda a: np.ascontiguousarray(np.asarray(a, dtype=np.float32))
    x = f32(x).reshape(NCORE, BL, T, D)
    shared = dict(
        Wq=f32(Wq), Wk=f32(Wk), Wv=f32(Wv), Wo=f32(Wo), bo=f32(bo),
        rel_k_table=f32(rel_k_table), rel_v_table=f32(rel_v_table),
    )
    return [dict(x=np.ascontiguousarray(x[i]), **shared) for i in range(NCORE)]


def kernel(x, Wq, Wk, Wv, Wo, bo, rel_k_table, rel_v_table):
    global _NC_CACHE
    if _NC_CACHE is None:
        _NC_CACHE = build()
    in_maps = _make_in_maps(x, Wq, Wk, Wv, Wo, bo, rel_k_table, rel_v_table)
    res = run_bass_kernel_spmd(_NC_CACHE, in_maps, list(range(NCORE)))
    out = np.concatenate([res.results[i]["y"] for i in range(NCORE)], axis=0)
    return out.reshape(B, T, D).astype(np.float32)


# revision 20
# speedup vs baseline: 2.1640x; 2.1640x over previous
"""Trainium2 Bass kernel for BasicTransformerBlock_Epipolar (relative-position attention).

Math (per batch b, head h):
  q = x@Wq, k = x@Wk, v = x@Wv            (H=16 heads, dh=64)
  sim[t,s]  = (q[t]·k[s] + q[t]·Tk[s-t+1024]) * dh^-0.5
  attn      = softmax_s(sim)
  out[t]    = sum_s attn[t,s]*v[s] + sum_s attn[t,s]*Tv[s-t+1024]
  y = out@Wo + bo

Sharding: data-parallel over batch, 2 batches per core, 8 cores, no collectives.

Dtypes: fp32r (rounded fp32, ~1e-4 rel) for all the fat matmuls; bf16 for the
attention-weight path (attn@v, rel-v) where weights are in [0, e^5] and the
rel tables are small corrections; f32 accumulation everywhere (PSUM).

The relative-position terms need a "skew" (diagonal remap) which SBUF access
patterns cannot express; both are routed through DRAM with strided access
patterns:
  - R[t,r] = q[t]·Tk[r] is computed blockwise as a plain matmul, written to a
    row-stride-2049 buffer (bf16), and read back with row stride 2048, which
    yields exactly M[t,s] = R[t, s-t+1024].
  - e = exp(SCALE*sim) (unnormalized attn) is written contiguously (row
    stride 1024, bf16) and read back with row stride 1025, which yields
    A_skew[i,j] = e[i, i+j-127]; PE-transposed chunks of it contract with Tv
    into the same PSUM as attn@v.  Softmax normalization is deferred to the
    PSUM eviction (one multiply by a broadcast 1/l row).
"""

import sys

sys.path.insert(0, "/opt/trn_rl_repo")

import numpy as np

import concourse.bass as bass
import concourse.tile as tile
from concourse import bacc, mybir
from concourse.bass_utils import run_bass_kernel_spmd
from concourse.masks import make_identity

FP = mybir.dt.float32
FR = mybir.dt.float32r
BF = mybir.dt.bfloat16

B, T, D = 16, 1024, 1024
H, DH = 16, 64
NCORE = 8
BL = B // NCORE          # batches per core
TL = BL * T              # local token rows
SCALE = DH ** -0.5
NHP = H // 2             # head pairs
NBLK = T // 128          # 128-row blocks per batch
WREL = 1151              # rel window width per 128 t-block
RSTRIDE = 2049
RSEG = T * RSTRIDE       # rbuf elements per (b, h)
AGUARD = 128
ASEG = AGUARD + 128 * 1024 + AGUARD  # abuf elements per (b, h, blk)


def _ap(t_ap, offset, pattern):
    return bass.AP(tensor=t_ap.tensor, offset=offset, ap=pattern)


def build(num_b=BL, num_hp=NHP, num_blk=NBLK, dbg=False):
    nc = bacc.Bacc("TRN2", target_bir_lowering=False, debug=False, num_devices=NCORE)

    x = nc.dram_tensor("x", [BL, T, D], FP, kind="ExternalInput").ap()
    wq = nc.dram_tensor("Wq", [D, D], FP, kind="ExternalInput").ap()
    wk = nc.dram_tensor("Wk", [D, D], FP, kind="ExternalInput").ap()
    wv = nc.dram_tensor("Wv", [D, D], FP, kind="ExternalInput").ap()
    wo = nc.dram_tensor("Wo", [D, D], FP, kind="ExternalInput").ap()
    bo = nc.dram_tensor("bo", [D], FP, kind="ExternalInput").ap()
    tk = nc.dram_tensor("rel_k_table", [2 * T + 1, DH], FP, kind="ExternalInput").ap()
    tv = nc.dram_tensor("rel_v_table", [2 * T + 1, DH], FP, kind="ExternalInput").ap()
    y = nc.dram_tensor("y", [BL, T, D], FP, kind="ExternalOutput").ap()

    qT = nc.dram_tensor("qT", [D, TL], FR).ap()
    kT = nc.dram_tensor("kT", [D, TL], FR).ap()
    vB = nc.dram_tensor("vB", [TL, D], BF).ap()
    aot = nc.dram_tensor("aot", [D, TL], FR).ap()
    rbuf = nc.dram_tensor("rbuf", [BL * H * RSEG], BF).ap()
    abuf = nc.dram_tensor("abuf", [BL * H * NBLK * ASEG], BF).ap()
    dbgt = {}
    if dbg:
        for name, shape, dt in (
            ("dbg_r", [128, WREL], BF), ("dbg_msk", [128, T], BF),
            ("dbg_sims", [128, T], FP), ("dbg_abf", [128, T], BF),
            ("dbg_askw", [128, 1152], BF), ("dbg_l", [128, 1], FP),
            ("dbg_rec", [1, 128], FP), ("dbg_rb", [128, 128], FP),
            ("dbg_at", [128, 8, 128], BF), ("dbg_askt", [128, 9, 128], BF),
            ("dbg_ot", [128, 128], FP), ("dbg_aot", [128, 128], FR),
        ):
            dbgt[name] = nc.dram_tensor(name, shape, dt, kind="ExternalOutput").ap()

    with tile.TileContext(nc) as tc:
        const = tc.alloc_tile_pool(name="const", bufs=1)
        ps512 = tc.alloc_tile_pool(name="ps512", bufs=3, space="PSUM")
        pst = tc.alloc_tile_pool(name="pst", bufs=4, space="PSUM")

        # ---- constants ----
        ident = const.tile([128, 128], FP, tag="ident")
        make_identity(nc, ident)
        ident_bf = const.tile([128, 128], BF, tag="ident_bf")
        nc.vector.tensor_copy(ident_bf, ident)

        ones_i8 = const.tile([128, 128], mybir.dt.int8, tag="ones_i8")
        nc.vector.memset(ones_i8, 1)
        zeros_bf = const.tile([128, 128], BF, tag="zeros_bf")
        nc.vector.memset(zeros_bf, 0.0)
        # mask_lo[p,f] = 1 if p+f >= 127 ; mask_hi[p,f] = 1 if p+f <= 126
        # invalid corners of the skewed-attn read: chunk 0 is invalid where
        # p+f <= 126 (use mask_hi to zero), chunk 8 invalid where p+f >= 127.
        mask_lo = const.tile([128, 128], mybir.dt.int8, tag="mask_lo")
        nc.gpsimd.affine_select(
            out=mask_lo, in_=ones_i8, pattern=[[1, 128]],
            compare_op=mybir.AluOpType.is_ge, fill=0, base=-127,
            channel_multiplier=1,
        )
        mask_hi = const.tile([128, 128], mybir.dt.int8, tag="mask_hi")
        nc.gpsimd.affine_select(
            out=mask_hi, in_=ones_i8, pattern=[[-1, 128]],
            compare_op=mybir.AluOpType.is_ge, fill=0, base=126,
            channel_multiplier=-1,
        )

        # bo broadcast to all partitions
        bo128 = const.tile([128, D], FP, tag="bo128")
        nc.sync.dma_start(out=bo128, in_=_ap(bo, 0, [[0, 128], [1, D]]))

        # Tk^T resident in SBUF (f32r), duplicated across both partition
        # halves so it can feed row-packed matmuls for either head.
        tk_tmp = const.tile([128, 16, DH], FP, tag="tk_tmp")
        nc.sync.dma_start(
            out=tk_tmp, in_=tk[0 : 16 * 128, :].rearrange("(c p) d -> p c d", p=128)
        )
        tkT = const.tile([128, 16 * 128 + 4], FR, tag="tkT")
        for c in range(16):
            ptile = pst.tile([128, 128], FP, tag="pst")
            nc.tensor.transpose(ptile[0:DH, :], tk_tmp[:, c, :], ident)
            nc.scalar.copy(out=tkT[0:DH, c * 128 : (c + 1) * 128], in_=ptile[0:DH, :])
            nc.scalar.copy(out=tkT[DH:128, c * 128 : (c + 1) * 128], in_=ptile[0:DH, :])

        nc.scalar.copy(out=tkT[:, 16 * 128 : 16 * 128 + 4], in_=zeros_bf[:, 0:4])

        # Tv rows 1..2048 as bf16 chunks: tv_bf[p, m, d] = tv[1 + 128m + p, d]
        tv_tmp = const.tile([128, 16, DH], FP, tag="tv_tmp")
        nc.sync.dma_start(
            out=tv_tmp, in_=tv[1 : 1 + 16 * 128, :].rearrange("(m p) d -> p m d", p=128)
        )
        tv_bf = const.tile([128, 16, DH], BF, tag="tv_bf")
        nc.scalar.copy(out=tv_bf, in_=tv_tmp)

        # ================= Phase A: projections =================
        pa = tc.alloc_tile_pool(name="pa", bufs=2)
        pev = tc.alloc_tile_pool(name="pev", bufs=4)

        # x^T resident (f32r): xT[p, ic, tg] = x[tg//T, tg%T, 128*ic + p]
        xT = pa.tile([128, 8, TL], FR, tag="xT", bufs=1)
        for b in range(num_b):
            for tb in range(8):
                xt = pa.tile([128, D], FP, tag="xt")
                nc.sync.dma_start(out=xt, in_=x[b, tb * 128 : (tb + 1) * 128, :])
                for ic in range(8):
                    ptile = pst.tile([128, 128], FP, tag="pst")
                    nc.tensor.transpose(ptile, xt[:, ic * 128 : (ic + 1) * 128], ident)
                    nc.scalar.copy(
                        out=xT[:, ic, (b * 8 + tb) * 128 : (b * 8 + tb + 1) * 128],
                        in_=ptile,
                    )

        # q^T, k^T  (dst[j, tg] = sum_i W[i, j] * x[tg, i]) -> f32r DRAM
        for w_ap, dst in ((wq, qT), (wk, kT)):
            wld = pa.tile([128, 8, D], FP, tag="wld", bufs=1)
            nc.sync.dma_start(out=wld, in_=w_ap.rearrange("(c p) j -> p c j", p=128))
            wsb = pa.tile([128, 8, D], FR, tag="wsb")
            nc.scalar.copy(out=wsb, in_=wld)
            for jt in range(8):
                for tt in range(num_b * 2):
                    ps = ps512.tile([128, 512], FP, tag="ps512")
                    for ic in range(8):
                        nc.tensor.matmul(
                            ps,
                            lhsT=wsb[:, ic, jt * 128 : (jt + 1) * 128],
                            rhs=xT[:, ic, tt * 512 : (tt + 1) * 512],
                            start=(ic == 0),
                            stop=(ic == 7),
                        )
                    ev = pev.tile([128, 512], FR, tag="ev")
                    nc.scalar.copy(out=ev, in_=ps)
                    nc.sync.dma_start(
                        out=dst[jt * 128 : (jt + 1) * 128, tt * 512 : (tt + 1) * 512],
                        in_=ev,
                    )

        # v (natural layout, bf16): vB[tg, j] = sum_i x[tg, i] * Wv[i, j]
        wld = pa.tile([128, 8, D], FP, tag="wld", bufs=1)
        nc.sync.dma_start(out=wld, in_=wv.rearrange("(c p) j -> p c j", p=128))
        wsb = pa.tile([128, 8, D], FR, tag="wsb")
        nc.scalar.copy(out=wsb, in_=wld)
        for tt in range(num_b * 8):
            for jh in range(2):
                ps = ps512.tile([128, 512], FP, tag="ps512")
                for ic in range(8):
                    nc.tensor.matmul(
                        ps,
                        lhsT=xT[:, ic, tt * 128 : (tt + 1) * 128],
                        rhs=wsb[:, ic, jh * 512 : (jh + 1) * 512],
                        start=(ic == 0),
                        stop=(ic == 7),
                    )
                ev = pev.tile([128, 512], BF, tag="evb")
                nc.scalar.copy(out=ev, in_=ps)
                nc.sync.dma_start(
                    out=vB[tt * 128 : (tt + 1) * 128, jh * 512 : (jh + 1) * 512],
                    in_=ev,
                )

        pev.release()
        pa.release()

        pst.release()

        # ================= Phase B: attention =================
        pb = tc.alloc_tile_pool(name="pb", bufs=2)
        pstl = tc.alloc_tile_pool(name="pstl", bufs=1, space="PSUM")
        pstb = tc.alloc_tile_pool(name="pstb", bufs=2, space="PSUM")
        pso = tc.alloc_tile_pool(name="pso", bufs=2, space="PSUM")

        def stage1(b, hp, k, qk, kt, vv):
            """R matmuls + skew bounce + sim + exp (unnormalized, bf16) +
            1/l broadcast row.  Returns context for stage2."""
            t0 = 128 * k
            r0 = 897 - t0
            abfs = []
            segs = []
            recs = pb.tile([1, 256], FP, tag="recs")
            rb128 = pb.tile([128, 256], FP, tag="rb128")
            for h in (0, 1):
                hg = 2 * hp + h
                bh_base = (b * H + hg) * RSEG
                seg = ((b * H + hg) * NBLK + k) * ASEG
                segs.append(seg)
                tp = (64 * h, 0)
                lhs_q = qk[64 * h : 64 * h + 64, t0 : t0 + 128]

                # R[t, r] = q[t]·Tk[r] over the block window, bounced via DRAM
                rsb = pb.tile([128, WREL], BF, tag=f"rsb{h}")
                for c0, cw in ((0, 512), (512, 512), (1024, 127)):
                    cm = 128 if cw == 127 else cw
                    ps = ps512.tile([128, 512], FP, tag="ps512")
                    nc.tensor.matmul(
                        ps[:, 0:cm],
                        lhsT=lhs_q,
                        rhs=tkT[64 * h : 64 * h + 64, r0 + c0 : r0 + c0 + cm],
                        start=True,
                        stop=True,
                        tile_position=tp,
                    )
                    nc.scalar.copy(out=rsb[:, c0 : c0 + cw], in_=ps[:, 0:cw])
                nc.scalar.dma_start(
                    out=_ap(rbuf, bh_base + t0 * 2048 + 897, [[2049, 128], [1, WREL]]),
                    in_=rsb,
                )

                # skewed read-back: msk[i, s] = R[t0+i, s - (t0+i) + 1024]
                msk = pb.tile([128, T], BF, tag=f"msk{h}")
                nc.sync.dma_start(
                    out=msk,
                    in_=_ap(rbuf, bh_base + t0 * 2048 + 1024, [[2048, 128], [1, T]]),
                )

                # sim = q@k^T + msk ; e = exp(SCALE*sim) (bf16), l = rowsum(e)
                sims = pb.tile([128, T], FP, tag=f"sims{h}")
                for n in range(2):
                    ps = ps512.tile([128, 512], FP, tag="ps512")
                    nc.tensor.matmul(
                        ps,
                        lhsT=lhs_q,
                        rhs=kt[64 * h : 64 * h + 64, n * 512 : (n + 1) * 512],
                        start=True,
                        stop=True,
                        tile_position=tp,
                    )
                    nc.vector.tensor_add(
                        sims[:, n * 512 : (n + 1) * 512], ps, msk[:, n * 512 : (n + 1) * 512]
                    )
                lsum = pb.tile([128, 1], FP, tag=f"lsum{h}")
                abf = pb.tile([128, T], BF, tag=f"abf{h}")
                nc.scalar.activation(
                    out=abf, in_=sims, func=mybir.ActivationFunctionType.Exp,
                    scale=float(SCALE), accum_out=lsum,
                )
                nc.scalar.dma_start(
                    out=_ap(abuf, seg + AGUARD, [[1024, 128], [1, 1024]]), in_=abf
                )
                abfs.append(abf)

                if dbg and b == 0 and hp == 0 and k == 0 and h == 0:
                    nc.sync.dma_start(out=dbgt["dbg_r"], in_=rsb)
                    nc.sync.dma_start(out=dbgt["dbg_msk"], in_=msk)
                    nc.sync.dma_start(out=dbgt["dbg_sims"], in_=sims)
                    nc.sync.dma_start(out=dbgt["dbg_abf"], in_=abf)
                    nc.sync.dma_start(out=dbgt["dbg_l"], in_=lsum)

                # 1/l as a broadcast row into rb128[64h:64h+64, :]
                pl = pstl.tile([128, 128], FP, tag="pstl")
                nc.tensor.transpose(pl[0:1, :], lsum, ident)
                nc.vector.reciprocal(recs[0:1, 128 * h : 128 * h + 128], pl[0:1, :])
                if dbg and b == 0 and hp == 0 and k == 0 and h == 0:
                    nc.sync.dma_start(out=dbgt["dbg_rec"], in_=recs[0:1, 0:128])
            nc.gpsimd.partition_broadcast(rb128, recs)
            if dbg and b == 0 and hp == 0 and k == 0:
                nc.sync.dma_start(out=dbgt["dbg_rb"], in_=rb128[:, 0:128])
            return dict(b=b, hp=hp, k=k, vv=vv, abfs=abfs, segs=segs, rb128=rb128)

        def stage2(b, hp, k, vv, abfs, segs, rb128):
            """e^T via PE transpose + attn@v + rel_v, accumulate out^T in
            PSUM, normalize by 1/l on evict, write to aot (f32r)."""
            askws = []
            for h in (0, 1):
                # wide skewed read of e: askw[i, j] = e[i, i + j - 127]
                askw = pb.tile([128, 1152], BF, tag=f"askw{h}")
                nc.sync.dma_start(
                    out=askw, in_=_ap(abuf, segs[h] + 1, [[1025, 128], [1, 1152]])
                )
                # zero the invalid skew corners in place
                nc.vector.copy_predicated(askw[:, 0:128], mask_hi, zeros_bf)
                nc.vector.copy_predicated(askw[:, 1024:1152], mask_lo, zeros_bf)
                askws.append(askw)

            po = pso.tile([128, 128], FP, tag="po")
            for h in (0, 1):
                at = pb.tile([128, 8, 128], BF, tag=f"at{h}")
                for c in range(8):
                    ptile = pstb.tile([128, 128], BF, tag="pstb")
                    nc.tensor.transpose(
                        ptile, abfs[h][:, c * 128 : (c + 1) * 128], ident_bf
                    )
                    nc.vector.tensor_copy(at[:, c, :], ptile)
                if dbg and b == 0 and hp == 0 and k == 0 and h == 0:
                    nc.sync.dma_start(out=dbgt["dbg_at"], in_=at)
                for c in range(8):
                    nc.tensor.matmul(
                        po[64 * h : 64 * h + 64, :],
                        lhsT=vv[:, c, 64 * h : 64 * h + 64],
                        rhs=at[:, c, :],
                        start=(c == 0),
                        stop=False,
                        tile_position=(0, 64 * h),
                    )
                askt = pb.tile([128, 9, 128], BF, tag=f"askt{h}")
                for c in range(9):
                    ptile = pstb.tile([128, 128], BF, tag="pstb")
                    nc.tensor.transpose(
                        ptile, askws[h][:, c * 128 : (c + 1) * 128], ident_bf
                    )
                    nc.vector.tensor_copy(askt[:, c, :], ptile)
                if dbg and b == 0 and hp == 0 and k == 0 and h == 0:
                    nc.sync.dma_start(out=dbgt["dbg_askt"], in_=askt)
                for c in range(9):
                    nc.tensor.matmul(
                        po[64 * h : 64 * h + 64, :],
                        lhsT=tv_bf[:, 7 - k + c, :],
                        rhs=askt[:, c, :],
                        start=False,
                        stop=(c == 8),
                        tile_position=(0, 64 * h),
                    )
            if dbg and b == 0 and hp == 0 and k == 0:
                nc.sync.dma_start(out=dbgt["dbg_askw"], in_=askws[0])
            ot = pb.tile([128, 128], FR, tag="ot")
            nc.vector.tensor_mul(ot[0:64, :], po[0:64, :], rb128[0:64, 0:128])
            nc.vector.tensor_mul(ot[64:128, :], po[64:128, :], rb128[64:128, 128:256])
            if dbg and b == 0 and hp == 0 and k == 0:
                otf = pb.tile([128, 128], FP, tag="otf")
                nc.vector.tensor_copy(otf, po)
                nc.sync.dma_start(out=dbgt["dbg_ot"], in_=otf)
            nc.sync.dma_start(
                out=aot[128 * hp : 128 * (hp + 1), b * T + 128 * k : b * T + 128 * (k + 1)],
                in_=ot,
            )

        pending = None
        for b in range(num_b):
            for hp in range(num_hp):
                qk = pb.tile([128, T], FR, tag="qk")
                nc.sync.dma_start(
                    out=qk, in_=qT[128 * hp : 128 * (hp + 1), b * T : (b + 1) * T]
                )
                kt = pb.tile([128, T], FR, tag="kt")
                nc.sync.dma_start(
                    out=kt, in_=kT[128 * hp : 128 * (hp + 1), b * T : (b + 1) * T]
                )
                vv = pb.tile([128, 8, 128], BF, tag="vv")
                nc.sync.dma_start(
                    out=vv,
                    in_=vB[b * T : (b + 1) * T, 128 * hp : 128 * (hp + 1)].rearrange(
                        "(c p) d -> p c d", p=128
                    ),
                )
                for k in range(num_blk):
                    cur = stage1(b, hp, k, qk, kt, vv)
                    if pending is not None:
                        stage2(**pending)
                    pending = cur
        if pending is not None:
            stage2(**pending)
        pb.release()
        pso.release()
        pstb.release()
        pstl.release()

        if dbg:
            dsb = pb2 = tc.alloc_tile_pool(name="pdbg", bufs=1)
            dt_ = dsb.tile([128, 128], FR, tag="dt_")
            nc.sync.dma_start(out=dt_, in_=aot[0:128, 0:128])
            nc.sync.dma_start(out=dbgt["dbg_aot"], in_=dt_)
            dsb.release()

        # ================= Phase C: output projection =================
        pc = tc.alloc_tile_pool(name="pc", bufs=2)
        wld_o = pc.tile([128, 8, D], FP, tag="wld_o", bufs=1)
        nc.sync.dma_start(out=wld_o, in_=wo.rearrange("(c p) j -> p c j", p=128))
        wsb_o = pc.tile([128, 8, D], FR, tag="wsb_o", bufs=1)
        nc.scalar.copy(out=wsb_o, in_=wld_o)
        for tt in range(num_b * 8):
            asb = pc.tile([128, 8, 128], FR, tag="asb")
            nc.sync.dma_start(
                out=asb,
                in_=aot[:, tt * 128 : (tt + 1) * 128].rearrange(
                    "(c p) t -> p c t", p=128
                ),
            )
            for eh in range(2):
                ps = ps512.tile([128, 512], FP, tag="ps512")
                for ic in range(8):
                    nc.tensor.matmul(
                        ps,
                        lhsT=asb[:, ic, :],
                        rhs=wsb_o[:, ic, eh * 512 : (eh + 1) * 512],
                        start=(ic == 0),
                        stop=(ic == 7),
                    )
                ysb = pc.tile([128, 512], FP, tag="ysb")
                nc.vector.tensor_add(ysb, ps, bo128[:, eh * 512 : (eh + 1) * 512])
                nc.sync.dma_start(
                    out=y[tt // 8, (tt % 8) * 128 : (tt % 8 + 1) * 128,
                          eh * 512 : (eh + 1) * 512],
                    in_=ysb,
                )
        pc.release()

        ps512.release()
        const.release()

    nc.compile()
    return nc


_NC_CACHE = None


def _make_in_maps(x, Wq, Wk, Wv, Wo, bo, rel_k_table, rel_v_table):
    f32 = lambda a: np.ascontiguousarray(np.asarray(a, dtype=np.float32))
    x = f32(x).reshape(NCORE, BL, T, D)
    shared = dict(
        Wq=f32(Wq), Wk=f32(Wk), Wv=f32(Wv), Wo=f32(Wo), bo=f32(bo),
        rel_k_table=f32(rel_k_table), rel_v_table=f32(rel_v_table),
    )
    return [dict(x=np.ascontiguousarray(x[i]), **shared) for i in range(NCORE)]


def kernel(x, Wq, Wk, Wv, Wo, bo, rel_k_table, rel_v_table):
    global _NC_CACHE
    if _NC_CACHE is None:
        _NC_CACHE = build()
    in_maps = _make_in_maps(x, Wq, Wk, Wv, Wo, bo, rel_k_table, rel_v_table)
    res = run_bass_kernel_spmd(_NC_CACHE, in_maps, list(range(NCORE)))
    out = np.concatenate([res.results[i]["y"] for i in range(NCORE)], axis=0)
    return out.reshape(B, T, D).astype(np.float32)


# revision 21
# speedup vs baseline: 2.7697x; 1.2799x over previous
"""Trainium2 Bass kernel for BasicTransformerBlock_Epipolar (relative-position attention).

Math (per batch b, head h):
  q = x@Wq, k = x@Wk, v = x@Wv            (H=16 heads, dh=64)
  sim[t,s]  = (q[t]·k[s] + q[t]·Tk[s-t+1024]) * dh^-0.5
  attn      = softmax_s(sim)
  out[t]    = sum_s attn[t,s]*v[s] + sum_s attn[t,s]*Tv[s-t+1024]
  y = out@Wo + bo

Sharding: data-parallel over batch, 2 batches per core, 8 cores, no collectives.

Dtypes: fp32r (rounded fp32, ~1e-4 rel) for all the fat matmuls; bf16 for the
attention-weight path (attn@v, rel-v) where weights are in [0, e^5] and the
rel tables are small corrections; f32 accumulation everywhere (PSUM).

The relative-position terms need a "skew" (diagonal remap) which SBUF access
patterns cannot express; both are routed through DRAM with strided access
patterns:
  - R[t,r] = q[t]·Tk[r] is computed blockwise as a plain matmul, written to a
    row-stride-2049 buffer (bf16), and read back with row stride 2048, which
    yields exactly M[t,s] = R[t, s-t+1024].
  - e = exp(SCALE*sim) (unnormalized attn) is written contiguously (row
    stride 1024, bf16) and read back with row stride 1025, which yields
    A_skew[i,j] = e[i, i+j-127]; PE-transposed chunks of it contract with Tv
    into the same PSUM as attn@v.  Softmax normalization is deferred to the
    PSUM eviction (one multiply by a broadcast 1/l row).
"""

import sys

sys.path.insert(0, "/opt/trn_rl_repo")

import numpy as np

import concourse.bass as bass
import concourse.tile as tile
from concourse import bacc, mybir
from concourse.bass_utils import run_bass_kernel_spmd
from concourse.masks import make_identity

FP = mybir.dt.float32
FR = mybir.dt.float32r
BF = mybir.dt.bfloat16

B, T, D = 16, 1024, 1024
H, DH = 16, 64
NCORE = 8
BL = B // NCORE          # batches per core
TL = BL * T              # local token rows
SCALE = DH ** -0.5
NHP = H // 2             # head pairs
NBLK = T // 128          # 128-row blocks per batch
WREL = 1151              # rel window width per 128 t-block
RSTRIDE = 2049
RSEG = T * RSTRIDE       # rbuf elements per (b, h)
AGUARD = 128
ASEG = AGUARD + 128 * 1024 + AGUARD  # abuf elements per (b, h, blk)


def _ap(t_ap, offset, pattern):
    return bass.AP(tensor=t_ap.tensor, offset=offset, ap=pattern)


def build(num_b=BL, num_hp=NHP, num_blk=NBLK, dbg=False):
    nc = bacc.Bacc("TRN2", target_bir_lowering=False, debug=False, num_devices=NCORE)

    x = nc.dram_tensor("x", [BL, T, D], FP, kind="ExternalInput").ap()
    wq = nc.dram_tensor("Wq", [D, D], FP, kind="ExternalInput").ap()
    wk = nc.dram_tensor("Wk", [D, D], FP, kind="ExternalInput").ap()
    wv = nc.dram_tensor("Wv", [D, D], FP, kind="ExternalInput").ap()
    wo = nc.dram_tensor("Wo", [D, D], FP, kind="ExternalInput").ap()
    bo = nc.dram_tensor("bo", [D], FP, kind="ExternalInput").ap()
    tk = nc.dram_tensor("rel_k_table", [2 * T + 1, DH], FP, kind="ExternalInput").ap()
    tv = nc.dram_tensor("rel_v_table", [2 * T + 1, DH], FP, kind="ExternalInput").ap()
    y = nc.dram_tensor("y", [BL, T, D], FP, kind="ExternalOutput").ap()

    qT = nc.dram_tensor("qT", [D, TL], FR).ap()
    kT = nc.dram_tensor("kT", [D, TL], FR).ap()
    vB = nc.dram_tensor("vB", [TL, D], BF).ap()
    aot = nc.dram_tensor("aot", [D, TL], FR).ap()
    rbuf = nc.dram_tensor("rbuf", [BL * H * RSEG], BF).ap()
    abuf = nc.dram_tensor("abuf", [BL * H * NBLK * ASEG], BF).ap()
    dbgt = {}
    if dbg:
        for name, shape, dt in (
            ("dbg_r", [128, WREL], BF), ("dbg_msk", [128, T], BF),
            ("dbg_sims", [128, T], FP), ("dbg_abf", [128, T], BF),
            ("dbg_askw", [128, 1152], BF), ("dbg_l", [128, 1], FP),
            ("dbg_rec", [1, 128], FP), ("dbg_rb", [128, 128], FP),
            ("dbg_at", [128, 8, 128], BF), ("dbg_askt", [128, 9, 128], BF),
            ("dbg_ot", [128, 128], FP), ("dbg_aot", [128, 128], FR),
        ):
            dbgt[name] = nc.dram_tensor(name, shape, dt, kind="ExternalOutput").ap()

    with tile.TileContext(nc) as tc:
        const = tc.alloc_tile_pool(name="const", bufs=1)
        ps512 = tc.alloc_tile_pool(name="ps512", bufs=3, space="PSUM")
        pst = tc.alloc_tile_pool(name="pst", bufs=4, space="PSUM")

        # ---- constants ----
        ident = const.tile([128, 128], FP, tag="ident")
        make_identity(nc, ident)
        ident_bf = const.tile([128, 128], BF, tag="ident_bf")
        nc.vector.tensor_copy(ident_bf, ident)

        ones_i8 = const.tile([128, 128], mybir.dt.int8, tag="ones_i8")
        nc.vector.memset(ones_i8, 1)
        zeros_bf = const.tile([128, 128], BF, tag="zeros_bf")
        nc.vector.memset(zeros_bf, 0.0)
        # mask_lo[p,f] = 1 if p+f >= 127 ; mask_hi[p,f] = 1 if p+f <= 126
        # invalid corners of the skewed-attn read: chunk 0 is invalid where
        # p+f <= 126 (use mask_hi to zero), chunk 8 invalid where p+f >= 127.
        mask_lo = const.tile([128, 128], mybir.dt.int8, tag="mask_lo")
        nc.gpsimd.affine_select(
            out=mask_lo, in_=ones_i8, pattern=[[1, 128]],
            compare_op=mybir.AluOpType.is_ge, fill=0, base=-127,
            channel_multiplier=1,
        )
        mask_hi = const.tile([128, 128], mybir.dt.int8, tag="mask_hi")
        nc.gpsimd.affine_select(
            out=mask_hi, in_=ones_i8, pattern=[[-1, 128]],
            compare_op=mybir.AluOpType.is_ge, fill=0, base=126,
            channel_multiplier=-1,
        )

        # bo broadcast to all partitions
        bo128 = const.tile([128, D], FP, tag="bo128")
        nc.sync.dma_start(out=bo128, in_=_ap(bo, 0, [[0, 128], [1, D]]))

        # Tk^T resident in SBUF (f32r), duplicated across both partition
        # halves so it can feed row-packed matmuls for either head.
        tk_tmp = const.tile([128, 16, DH], FP, tag="tk_tmp")
        nc.sync.dma_start(
            out=tk_tmp, in_=tk[0 : 16 * 128, :].rearrange("(c p) d -> p c d", p=128)
        )
        tkT = const.tile([128, 16 * 128 + 4], FR, tag="tkT")
        for c in range(16):
            ptile = pst.tile([128, 128], FP, tag="pst")
            nc.tensor.transpose(ptile[0:DH, :], tk_tmp[:, c, :], ident)
            nc.scalar.copy(out=tkT[0:DH, c * 128 : (c + 1) * 128], in_=ptile[0:DH, :])
            nc.scalar.copy(out=tkT[DH:128, c * 128 : (c + 1) * 128], in_=ptile[0:DH, :])

        nc.scalar.copy(out=tkT[:, 16 * 128 : 16 * 128 + 4], in_=zeros_bf[:, 0:4])

        # Tv rows 1..2048 as bf16 chunks: tv_bf[p, m, d] = tv[1 + 128m + p, d]
        tv_tmp = const.tile([128, 16, DH], FP, tag="tv_tmp")
        nc.sync.dma_start(
            out=tv_tmp, in_=tv[1 : 1 + 16 * 128, :].rearrange("(m p) d -> p m d", p=128)
        )
        tv_bf = const.tile([128, 16, DH], BF, tag="tv_bf")
        nc.scalar.copy(out=tv_bf, in_=tv_tmp)

        # ================= Phase A: projections =================
        pa = tc.alloc_tile_pool(name="pa", bufs=2)
        pev = tc.alloc_tile_pool(name="pev", bufs=4)

        # x^T resident (f32r): xT[p, ic, tg] = x[tg//T, tg%T, 128*ic + p]
        xT = pa.tile([128, 8, TL], FR, tag="xT", bufs=1)
        for b in range(num_b):
            for tb in range(8):
                xt = pa.tile([128, D], FP, tag="xt")
                nc.sync.dma_start(out=xt, in_=x[b, tb * 128 : (tb + 1) * 128, :])
                for ic in range(8):
                    ptile = pst.tile([128, 128], FP, tag="pst")
                    nc.tensor.transpose(ptile, xt[:, ic * 128 : (ic + 1) * 128], ident)
                    nc.scalar.copy(
                        out=xT[:, ic, (b * 8 + tb) * 128 : (b * 8 + tb + 1) * 128],
                        in_=ptile,
                    )

        # q^T, k^T  (dst[j, tg] = sum_i W[i, j] * x[tg, i]) -> f32r DRAM
        for w_ap, dst in ((wq, qT), (wk, kT)):
            wld = pa.tile([128, 8, D], FP, tag="wld", bufs=1)
            nc.sync.dma_start(out=wld, in_=w_ap.rearrange("(c p) j -> p c j", p=128))
            wsb = pa.tile([128, 8, D], FR, tag="wsb")
            nc.scalar.copy(out=wsb, in_=wld)
            for jt in range(8):
                for tt in range(num_b * 2):
                    ps = ps512.tile([128, 512], FP, tag="ps512")
                    for ic in range(8):
                        nc.tensor.matmul(
                            ps,
                            lhsT=wsb[:, ic, jt * 128 : (jt + 1) * 128],
                            rhs=xT[:, ic, tt * 512 : (tt + 1) * 512],
                            start=(ic == 0),
                            stop=(ic == 7),
                        )
                    ev = pev.tile([128, 512], FR, tag="ev")
                    nc.scalar.copy(out=ev, in_=ps)
                    nc.sync.dma_start(
                        out=dst[jt * 128 : (jt + 1) * 128, tt * 512 : (tt + 1) * 512],
                        in_=ev,
                    )

        # v (natural layout, bf16): vB[tg, j] = sum_i x[tg, i] * Wv[i, j]
        wld = pa.tile([128, 8, D], FP, tag="wld", bufs=1)
        nc.sync.dma_start(out=wld, in_=wv.rearrange("(c p) j -> p c j", p=128))
        wsb = pa.tile([128, 8, D], FR, tag="wsb")
        nc.scalar.copy(out=wsb, in_=wld)
        for tt in range(num_b * 8):
            for jh in range(2):
                ps = ps512.tile([128, 512], FP, tag="ps512")
                for ic in range(8):
                    nc.tensor.matmul(
                        ps,
                        lhsT=xT[:, ic, tt * 128 : (tt + 1) * 128],
                        rhs=wsb[:, ic, jh * 512 : (jh + 1) * 512],
                        start=(ic == 0),
                        stop=(ic == 7),
                    )
                ev = pev.tile([128, 512], BF, tag="evb")
                nc.scalar.copy(out=ev, in_=ps)
                nc.sync.dma_start(
                    out=vB[tt * 128 : (tt + 1) * 128, jh * 512 : (jh + 1) * 512],
                    in_=ev,
                )

        pev.release()
        pa.release()

        pst.release()

        # ================= Phase B: attention =================
        pb = tc.alloc_tile_pool(name="pb", bufs=2)
        pstl = tc.alloc_tile_pool(name="pstl", bufs=1, space="PSUM")
        pstb = tc.alloc_tile_pool(name="pstb", bufs=3, space="PSUM")
        pso = tc.alloc_tile_pool(name="pso", bufs=1, space="PSUM")

        def stage1(b, hp, k, qk, kt, vv):
            """R matmuls + skew bounce + sim + exp (unnormalized, bf16) +
            1/l broadcast row.  Returns context for stage2."""
            t0 = 128 * k
            r0 = 897 - t0
            abfs = []
            segs = []
            recs = pb.tile([1, 256], FP, tag="recs")
            rb128 = pb.tile([128, 256], FP, tag="rb128")
            for h in (0, 1):
                hg = 2 * hp + h
                bh_base = (b * H + hg) * RSEG
                seg = ((b * H + hg) * NBLK + k) * ASEG
                segs.append(seg)
                tp = (64 * h, 0)
                lhs_q = qk[64 * h : 64 * h + 64, t0 : t0 + 128]

                # R[t, r] = q[t]·Tk[r] over the block window, bounced via DRAM
                rsb = pb.tile([128, WREL], BF, tag=f"rsb{h}")
                for c0, cw in ((0, 512), (512, 512), (1024, 127)):
                    cm = 128 if cw == 127 else cw
                    ps = ps512.tile([128, 512], FP, tag="ps512")
                    nc.tensor.matmul(
                        ps[:, 0:cm],
                        lhsT=lhs_q,
                        rhs=tkT[64 * h : 64 * h + 64, r0 + c0 : r0 + c0 + cm],
                        start=True,
                        stop=True,
                        tile_position=tp,
                    )
                    nc.scalar.copy(out=rsb[:, c0 : c0 + cw], in_=ps[:, 0:cw])
                nc.scalar.dma_start(
                    out=_ap(rbuf, bh_base + t0 * 2048 + 897, [[2049, 128], [1, WREL]]),
                    in_=rsb,
                )

                # skewed read-back: msk[i, s] = R[t0+i, s - (t0+i) + 1024]
                msk = pb.tile([128, T], BF, tag=f"msk{h}")
                nc.sync.dma_start(
                    out=msk,
                    in_=_ap(rbuf, bh_base + t0 * 2048 + 1024, [[2048, 128], [1, T]]),
                )

                # sim = q@k^T + msk ; e = exp(SCALE*sim) (bf16), l = rowsum(e)
                sims = pb.tile([128, T], FP, tag=f"sims{h}")
                for n in range(2):
                    ps = ps512.tile([128, 512], FP, tag="ps512")
                    nc.tensor.matmul(
                        ps,
                        lhsT=lhs_q,
                        rhs=kt[64 * h : 64 * h + 64, n * 512 : (n + 1) * 512],
                        start=True,
                        stop=True,
                        tile_position=tp,
                    )
                    nc.vector.tensor_add(
                        sims[:, n * 512 : (n + 1) * 512], ps, msk[:, n * 512 : (n + 1) * 512]
                    )
                lsum = pb.tile([128, 1], FP, tag=f"lsum{h}")
                abf = pb.tile([128, T], BF, tag=f"abf{h}")
                nc.scalar.activation(
                    out=abf, in_=sims, func=mybir.ActivationFunctionType.Exp,
                    scale=float(SCALE), accum_out=lsum,
                )
                nc.scalar.dma_start(
                    out=_ap(abuf, seg + AGUARD, [[1024, 128], [1, 1024]]), in_=abf
                )
                abfs.append(abf)

                if dbg and b == 0 and hp == 0 and k == 0 and h == 0:
                    nc.sync.dma_start(out=dbgt["dbg_r"], in_=rsb)
                    nc.sync.dma_start(out=dbgt["dbg_msk"], in_=msk)
                    nc.sync.dma_start(out=dbgt["dbg_sims"], in_=sims)
                    nc.sync.dma_start(out=dbgt["dbg_abf"], in_=abf)
                    nc.sync.dma_start(out=dbgt["dbg_l"], in_=lsum)

                # 1/l as a broadcast row into rb128[64h:64h+64, :]
                recl = pb.tile([128, 1], FP, tag=f"recl{h}")
                nc.vector.reciprocal(recl, lsum)
                pl = pstl.tile([128, 128], FP, tag="pstl")
                nc.tensor.transpose(pl[0:1, :], recl, ident)
                nc.vector.tensor_copy(recs[0:1, 128 * h : 128 * h + 128], pl[0:1, :])
                if dbg and b == 0 and hp == 0 and k == 0 and h == 0:
                    nc.sync.dma_start(out=dbgt["dbg_rec"], in_=recs[0:1, 0:128])
            nc.gpsimd.partition_broadcast(rb128, recs)
            if dbg and b == 0 and hp == 0 and k == 0:
                nc.sync.dma_start(out=dbgt["dbg_rb"], in_=rb128[:, 0:128])
            return dict(b=b, hp=hp, k=k, vv=vv, abfs=abfs, segs=segs, rb128=rb128)

        def stage2(b, hp, k, vv, abfs, segs, rb128):
            """e^T via PE transpose + attn@v + rel_v, accumulate out^T in
            PSUM, normalize by 1/l on evict, write to aot (f32r)."""
            askws = []
            for h in (0, 1):
                # wide skewed read of e: askw[i, j] = e[i, i + j - 127]
                askw = pb.tile([128, 1152], BF, tag=f"askw{h}")
                nc.sync.dma_start(
                    out=askw, in_=_ap(abuf, segs[h] + 1, [[1025, 128], [1, 1152]])
                )
                # zero the invalid skew corners in place
                nc.vector.copy_predicated(askw[:, 0:128], mask_hi, zeros_bf)
                nc.vector.copy_predicated(askw[:, 1024:1152], mask_lo, zeros_bf)
                askws.append(askw)

            po = pso.tile([128, 128], FP, tag="po")
            for h in (0, 1):
                at = pb.tile([128, 8, 128], BF, tag=f"at{h}")
                for g in range(2):
                    ptile = pstb.tile([128, 4, 128], BF, tag="pstb")
                    for c4 in range(4):
                        nc.tensor.transpose(
                            ptile[:, c4, :],
                            abfs[h][:, (4 * g + c4) * 128 : (4 * g + c4 + 1) * 128],
                            ident_bf,
                        )
                    nc.vector.tensor_copy(at[:, 4 * g : 4 * g + 4, :], ptile)
                if dbg and b == 0 and hp == 0 and k == 0 and h == 0:
                    nc.sync.dma_start(out=dbgt["dbg_at"], in_=at)
                for c in range(8):
                    nc.tensor.matmul(
                        po[64 * h : 64 * h + 64, :],
                        lhsT=vv[:, c, 64 * h : 64 * h + 64],
                        rhs=at[:, c, :],
                        start=(c == 0),
                        stop=False,
                        tile_position=(0, 64 * h),
                    )
                askt = pb.tile([128, 9, 128], BF, tag=f"askt{h}")
                for g in range(3):
                    gw = 4 if g < 2 else 1
                    ptile = pstb.tile([128, 4, 128], BF, tag="pstb")
                    for c4 in range(gw):
                        nc.tensor.transpose(
                            ptile[:, c4, :],
                            askws[h][:, (4 * g + c4) * 128 : (4 * g + c4 + 1) * 128],
                            ident_bf,
                        )
                    nc.vector.tensor_copy(
                        askt[:, 4 * g : 4 * g + gw, :], ptile[:, 0:gw, :]
                    )
                if dbg and b == 0 and hp == 0 and k == 0 and h == 0:
                    nc.sync.dma_start(out=dbgt["dbg_askt"], in_=askt)
                for c in range(9):
                    nc.tensor.matmul(
                        po[64 * h : 64 * h + 64, :],
                        lhsT=tv_bf[:, 7 - k + c, :],
                        rhs=askt[:, c, :],
                        start=False,
                        stop=(c == 8),
                        tile_position=(0, 64 * h),
                    )
            if dbg and b == 0 and hp == 0 and k == 0:
                nc.sync.dma_start(out=dbgt["dbg_askw"], in_=askws[0])
            ot = pb.tile([128, 128], FR, tag="ot")
            nc.vector.tensor_mul(ot[0:64, :], po[0:64, :], rb128[0:64, 0:128])
            nc.vector.tensor_mul(ot[64:128, :], po[64:128, :], rb128[64:128, 128:256])
            if dbg and b == 0 and hp == 0 and k == 0:
                otf = pb.tile([128, 128], FP, tag="otf")
                nc.vector.tensor_copy(otf, po)
                nc.sync.dma_start(out=dbgt["dbg_ot"], in_=otf)
            nc.sync.dma_start(
                out=aot[128 * hp : 128 * (hp + 1), b * T + 128 * k : b * T + 128 * (k + 1)],
                in_=ot,
            )

        pending = None
        for b in range(num_b):
            for hp in range(num_hp):
                qk = pb.tile([128, T], FR, tag="qk")
                nc.sync.dma_start(
                    out=qk, in_=qT[128 * hp : 128 * (hp + 1), b * T : (b + 1) * T]
                )
                kt = pb.tile([128, T], FR, tag="kt")
                nc.sync.dma_start(
                    out=kt, in_=kT[128 * hp : 128 * (hp + 1), b * T : (b + 1) * T]
                )
                vv = pb.tile([128, 8, 128], BF, tag="vv")
                nc.sync.dma_start(
                    out=vv,
                    in_=vB[b * T : (b + 1) * T, 128 * hp : 128 * (hp + 1)].rearrange(
                        "(c p) d -> p c d", p=128
                    ),
                )
                for k in range(num_blk):
                    cur = stage1(b, hp, k, qk, kt, vv)
                    if pending is not None:
                        stage2(**pending)
                    pending = cur
        if pending is not None:
            stage2(**pending)
        pb.release()
        pso.release()
        pstb.release()
        pstl.release()

        if dbg:
            dsb = pb2 = tc.alloc_tile_pool(name="pdbg", bufs=1)
            dt_ = dsb.tile([128, 128], FR, tag="dt_")
            nc.sync.dma_start(out=dt_, in_=aot[0:128, 0:128])
            nc.sync.dma_start(out=dbgt["dbg_aot"], in_=dt_)
            dsb.release()

        # ================= Phase C: output projection =================
        pc = tc.alloc_tile_pool(name="pc", bufs=2)
        wld_o = pc.tile([128, 8, D], FP, tag="wld_o", bufs=1)
        nc.sync.dma_start(out=wld_o, in_=wo.rearrange("(c p) j -> p c j", p=128))
        wsb_o = pc.tile([128, 8, D], FR, tag="wsb_o", bufs=1)
        nc.scalar.copy(out=wsb_o, in_=wld_o)
        for tt in range(num_b * 8):
            asb = pc.tile([128, 8, 128], FR, tag="asb")
            nc.sync.dma_start(
                out=asb,
                in_=aot[:, tt * 128 : (tt + 1) * 128].rearrange(
                    "(c p) t -> p c t", p=128
                ),
            )
            for eh in range(2):
                ps = ps512.tile([128, 512], FP, tag="ps512")
                for ic in range(8):
                    nc.tensor.matmul(
                        ps,
                        lhsT=asb[:, ic, :],
                        rhs=wsb_o[:, ic, eh * 512 : (eh + 1) * 512],
                        start=(ic == 0),
                        stop=(ic == 7),
                    )
                ysb = pc.tile([128, 512], FP, tag="ysb")
                nc.vector.tensor_add(ysb, ps, bo128[:, eh * 512 : (eh + 1) * 512])
                nc.sync.dma_start(
                    out=y[tt // 8, (tt % 8) * 128 : (tt % 8 + 1) * 128,
                          eh * 512 : (eh + 1) * 512],
                    in_=ysb,
                )
        pc.release()

        ps512.release()
        const.release()

    nc.compile()
    return nc


_NC_CACHE = None


def _make_in_maps(x, Wq, Wk, Wv, Wo, bo, rel_k_table, rel_v_table):
    f32 = lambda a: np.ascontiguousarray(np.asarray(a, dtype=np.float32))
    x = f32(x).reshape(NCORE, BL, T, D)
    shared = dict(
        Wq=f32(Wq), Wk=f32(Wk), Wv=f32(Wv), Wo=f32(Wo), bo=f32(bo),
        rel_k_table=f32(rel_k_table), rel_v_table=f32(rel_v_table),
    )
    return [dict(x=np.ascontiguousarray(x[i]), **shared) for i in range(NCORE)]


def kernel(x, Wq, Wk, Wv, Wo, bo, rel_k_table, rel_v_table):
    global _NC_CACHE
    if _NC_CACHE is None:
        _NC_CACHE = build()
    in_maps = _make_in_maps(x, Wq, Wk, Wv, Wo, bo, rel_k_table, rel_v_table)
    res = run_bass_kernel_spmd(_NC_CACHE, in_maps, list(range(NCORE)))
    out = np.concatenate([res.results[i]["y"] for i in range(NCORE)], axis=0)
    return out.reshape(B, T, D).astype(np.float32)
